# revision 1
# baseline (speedup 1.0000x reference)
"""BERT-base 12-layer encoder forward on 8 trn2 NeuronCores.

Strategy: pure data parallelism — batch B=8, one sequence per core, full
weights replicated, zero collectives. Per core all activations are kept
feature-major ([hidden, seq] with hidden on SBUF partitions) so every
projection is a PE matmul with the weight matrix as the stationary operand
in its natural HBM layout. All matmuls run in float32r (full PE rate,
~tf32+ precision). Softmax denominators come free from a ones-augmented
row in the PV matmul (M=65); per-token broadcasts (softmax recip, LN
mean/rstd) are K=1 ones-matmuls.
"""
import sys

sys.path.insert(0, "/opt/trn_rl_repo")

import numpy as np
import concourse.bass as bass
import concourse.mybir as mybir
import concourse.tile as tile
from concourse import bacc
from concourse.bass_utils import run_bass_kernel_spmd

F32 = mybir.dt.float32
F32R = mybir.dt.float32r
AF = mybir.ActivationFunctionType
ALU = mybir.AluOpType

L, H, NH, I = 12, 768, 12, 3072
DH = 64
B, S = 8, 512
KT = H // 128          # 6 k-tiles over hidden
MT = H // 128          # 6 m-tiles over hidden
IT = I // 128          # 24 tiles over intermediate
ST = S // 128          # 4 tiles over sequence
NP = NH // 2           # 6 head pairs
EPS = 1e-12
SCALE = 0.125          # 1/sqrt(64)


def build_program(repeat=1, n_layers=L, phases=("qkv", "att", "wo", "ffn")):
    nc = bacc.Bacc("TRN2", target_bir_lowering=False)

    XT = nc.dram_tensor("XT", [H, S], F32, kind="ExternalInput")
    EXTM = nc.dram_tensor("EXTM", [ST, 128], F32, kind="ExternalInput")
    WQ = nc.dram_tensor("WQ", [L, H, H], F32, kind="ExternalInput")
    WK = nc.dram_tensor("WK", [L, H, H], F32, kind="ExternalInput")
    WV = nc.dram_tensor("WV", [L, H, H], F32, kind="ExternalInput")
    WO = nc.dram_tensor("WO", [L, H, H], F32, kind="ExternalInput")
    WI = nc.dram_tensor("WI", [L, IT, 128, KT, 128], F32, kind="ExternalInput")
    WF = nc.dram_tensor("WF", [L, I, H], F32, kind="ExternalInput")
    BQ = nc.dram_tensor("BQ", [L, H], F32, kind="ExternalInput")
    BK = nc.dram_tensor("BK", [L, H], F32, kind="ExternalInput")
    BV = nc.dram_tensor("BV", [L, H], F32, kind="ExternalInput")
    BO = nc.dram_tensor("BO", [L, H], F32, kind="ExternalInput")
    BI = nc.dram_tensor("BI", [L, I], F32, kind="ExternalInput")
    BF = nc.dram_tensor("BF", [L, H], F32, kind="ExternalInput")
    G1 = nc.dram_tensor("G1", [L, H], F32, kind="ExternalInput")
    B1 = nc.dram_tensor("B1", [L, H], F32, kind="ExternalInput")
    G2 = nc.dram_tensor("G2", [L, H], F32, kind="ExternalInput")
    B2 = nc.dram_tensor("B2", [L, H], F32, kind="ExternalInput")
    OUT = nc.dram_tensor("OUT", [H, S], F32, kind="ExternalOutput")

    with tile.TileContext(nc) as tc:
        with (
            nc.allow_low_precision(reason="fp32r matmul pipeline"),
            tc.tile_pool(name="pers", bufs=1) as pers,
            tc.tile_pool(name="w768", bufs=14) as wpool,
            tc.tile_pool(name="wff1", bufs=4) as wf1pool,
            tc.tile_pool(name="sb", bufs=2) as sb,
        ):
            # ---- persistent activations ----
            xT = pers.tile([128, KT, S], F32R, tag="xT")
            nc.sync.dma_start(out=xT[:], in_=XT.ap().rearrange(
                "(k p) s -> p k s", p=128).bitcast(F32R))
            qT = pers.tile([128, KT, S], F32R, tag="qT")    # reused as ctxT
            kTt = pers.tile([128, KT, S], F32R, tag="kTt")  # reused as LN input y
            attnT = pers.tile([128, KT, S], F32R, tag="attnT")
            v_aug = pers.tile([128, ST, NH, DH + 1], F32R, tag="vaug")
            nc.vector.memset(v_aug[:, :, :, DH].bitcast(F32), 1.0)

            ext = pers.tile([128, ST], F32, tag="ext")
            nc.sync.dma_start(out=ext[:], in_=EXTM.ap().rearrange("k p -> p k"))

            # ---- constants ----
            ones64 = pers.tile([1, DH], F32R, tag="ones64")
            nc.vector.memset(ones64[:].bitcast(F32), 1.0)
            ones128c = pers.tile([1, 128], F32R, tag="ones128c")
            nc.vector.memset(ones128c[:].bitcast(F32), 1.0)
            ones128p = pers.tile([128, 1], F32R, tag="ones128p")
            nc.vector.memset(ones128p[:].bitcast(F32), 1.0)
            eps_t = pers.tile([1, 1], F32, tag="eps")
            nc.vector.memset(eps_t[:], EPS)

            # ---- per-layer params, loaded once (feature-major [128, L, KT]) ----
            def ppar(name, dram, kt):
                t = pers.tile([128, L, kt], F32, tag=name, name=name)
                nc.sync.dma_start(out=t[:], in_=dram.ap().rearrange(
                    "l (k p) -> p l k", p=128))
                return t
            bq_t = ppar("bq", BQ, KT); bk_t = ppar("bk", BK, KT)
            bo_t = ppar("bo", BO, KT); bf_t = ppar("bf", BF, KT)
            g1_t = ppar("g1", G1, KT); b1_t = ppar("b1", B1, KT)
            g2_t = ppar("g2", G2, KT); b2_t = ppar("b2", B2, KT)
            bi_t = ppar("bi", BI, IT)

            def layernorm(ps, y, gam, bet, l, out):
                """y: [128, KT, S] f32r post-residual; out[:, k, :] = LN(y)."""
                st_ps = ps.tile([1, 2, S], F32, tag="sum", name="sum")
                for k in range(KT):
                    nc.tensor.matmul(st_ps[:, 0, :], ones128p[:], y[:, k, :],
                                     start=(k == 0), stop=(k == KT - 1))
                for k in range(KT):
                    sq = sb.tile([128, S], F32R, tag="sq", name="sq", bufs=4)
                    nc.vector.tensor_mul(sq[:], y[:, k, :], y[:, k, :])
                    nc.tensor.matmul(st_ps[:, 1, :], ones128p[:], sq[:],
                                     start=(k == 0), stop=(k == KT - 1))
                mumsq = sb.tile([1, 2, S], F32R, tag="mu", name="mu")
                nc.vector.tensor_scalar_mul(mumsq[:], in0=st_ps[:], scalar1=1.0 / H)
                mu = mumsq[:, 0, :]
                msq = mumsq[:, 1, :]
                mu2 = sb.tile([1, S], F32, tag="mu2", name="mu2")
                nc.vector.tensor_mul(mu2[:], mu.bitcast(F32), mu.bitcast(F32))
                var = sb.tile([1, S], F32, tag="var", name="var")
                nc.vector.tensor_sub(var[:], msq.bitcast(F32), mu2[:])
                rstd = sb.tile([1, S], F32R, tag="rstd", name="rstd")
                nc.scalar.activation(rstd[:], var[:], AF.Sqrt, bias=eps_t[:])
                nc.vector.reciprocal(rstd[:], rstd[:])
                mub_ps = ps.tile([128, S], F32, tag="bcp", name="mub", bufs=2)
                nc.tensor.matmul(mub_ps[:], ones128c[:], mu, start=True, stop=True)
                rsb_ps = ps.tile([128, S], F32, tag="bcp", name="rsb", bufs=2)
                nc.tensor.matmul(rsb_ps[:], ones128c[:], rstd[:], start=True, stop=True)
                for k in range(KT):
                    t1 = sb.tile([128, S], F32, tag="lnt1", name="lnt1", bufs=3)
                    nc.vector.tensor_sub(t1[:], y[:, k, :].bitcast(F32), mub_ps[:])
                    t2 = sb.tile([128, S], F32, tag="lnt2", name="lnt2", bufs=3)
                    nc.vector.tensor_mul(t2[:], t1[:], rsb_ps[:])
                    nc.vector.scalar_tensor_tensor(
                        out[:, k, :], t2[:], gam[:, l, k:k + 1],
                        bet[:, l, k:k + 1].broadcast_to([128, S]),
                        op0=ALU.mult, op1=ALU.add)

            def load_w768(dram, l, k, name):
                w = wpool.tile([128, H], F32R, tag="w768", name=name)
                nc.sync.dma_start(
                    out=w[:], in_=dram.ap()[l, bass.ts(k, 128), :].bitcast(F32R))
                return w

            def layer_body(l):
                # ================= QKV projections =================
                if "qkv" not in phases:
                    return
                with tc.tile_pool(name="ps_qkv", bufs=1, space="PSUM") as ps:
                    wq = [load_w768(WQ, l, k, f"wq{k}") for k in range(KT)]
                    for m in range(MT):
                        p_q = ps.tile([128, S], F32, tag="qk", name="pq", bufs=4)
                        for k in range(KT):
                            nc.tensor.matmul(p_q[:], wq[k][:, bass.ts(m, 128)],
                                             xT[:, k, :], start=(k == 0),
                                             stop=(k == KT - 1))
                        nc.vector.tensor_scalar_add(qT[:, m, :], in0=p_q[:],
                                                    scalar1=bq_t[:, l, m:m + 1])
                    wk = [load_w768(WK, l, k, f"wk{k}") for k in range(KT)]
                    for m in range(MT):
                        p_k = ps.tile([128, S], F32, tag="qk", name="pk", bufs=4)
                        for k in range(KT):
                            nc.tensor.matmul(p_k[:], wk[k][:, bass.ts(m, 128)],
                                             xT[:, k, :], start=(k == 0),
                                             stop=(k == KT - 1))
                        nc.vector.tensor_scalar_add(kTt[:, m, :], in0=p_k[:],
                                                    scalar1=bk_t[:, l, m:m + 1])
                    # v, seq-major, bias via K=1 ones matmul
                    wv = [load_w768(WV, l, k, f"wv{k}") for k in range(KT)]
                    bv_row = sb.tile([1, H], F32R, tag="bvrow", name="bvrow", bufs=2)
                    nc.sync.dma_start(out=bv_row[:], in_=BV.ap()[l:l + 1, :].bitcast(F32R))
                    for s in range(ST):
                        p_a = ps.tile([128, S], F32, tag="va", name="pva", bufs=2)
                        p_b = ps.tile([128, 256], F32, tag="vb", name="pvb", bufs=2)
                        for k in range(KT):
                            nc.tensor.matmul(p_a[:], xT[:, k, bass.ts(s, 128)],
                                             wv[k][:, 0:512], start=(k == 0), stop=False)
                            nc.tensor.matmul(p_b[:], xT[:, k, bass.ts(s, 128)],
                                             wv[k][:, 512:768], start=(k == 0), stop=False)
                        nc.tensor.matmul(p_a[:], ones128c[:], bv_row[:, 0:512],
                                         start=False, stop=True)
                        nc.tensor.matmul(p_b[:], ones128c[:], bv_row[:, 512:768],
                                         start=False, stop=True)
                        nc.vector.tensor_copy(
                            v_aug[:, s, 0:8, 0:DH],
                            p_a[:].rearrange("p (h c) -> p h c", c=DH).bitcast(F32R))
                        nc.vector.tensor_copy(
                            v_aug[:, s, 8:12, 0:DH],
                            p_b[:].rearrange("p (h c) -> p h c", c=DH).bitcast(F32R))

                # ================= attention =================
                if "att" not in phases:
                    return
                with tc.tile_pool(name="ps_att", bufs=1, space="PSUM") as ps:
                    for pr in range(NP):
                        praw = sb.tile([128, S], F32R, tag="praw", name="praw", bufs=3)
                        bcp = sb.tile([128, S], F32, tag="bcpair", name="bcp", bufs=3)
                        for hh in range(2):
                            h = 2 * pr + hh
                            p0 = hh * DH
                            tp = None if hh == 0 else (64, 0)
                            exps = []
                            for m in range(ST):
                                s_ps = ps.tile([128, S], F32, tag="scores",
                                               name="sps", bufs=3)
                                nc.tensor.matmul(
                                    s_ps[:],
                                    kTt[p0:p0 + DH, pr, bass.ts(m, 128)],
                                    qT[p0:p0 + DH, pr, :],
                                    start=True, stop=True, tile_position=tp)
                                e_t = sb.tile([128, S], F32R, tag="exp",
                                              name="expt", bufs=7)
                                nc.scalar.activation(e_t[:], s_ps[:], AF.Exp,
                                                     bias=ext[:, m:m + 1], scale=SCALE)
                                exps.append(e_t)
                            c_ps = ps.tile([128, S], F32, tag="ctx", name="cps", bufs=3)
                            for m in range(ST):
                                nc.tensor.matmul(c_ps[0:DH + 1, :],
                                                 v_aug[:, m, h, :], exps[m][:],
                                                 start=(m == 0), stop=(m == ST - 1))
                            # raw ctx into pair rows (ACT copy, shifted for head B)
                            nc.vector.tensor_copy(praw[p0:p0 + DH, :].bitcast(F32),
                                                  c_ps[0:DH, :])
                            rcp = sb.tile([1, S], F32R, tag="rcp", name="rcp", bufs=4)
                            nc.vector.tensor_copy(rcp[:], c_ps[DH:DH + 1, :].bitcast(F32R))
                            nc.vector.reciprocal(rcp[:], rcp[:])
                            bc_ps = ps.tile([64, S], F32, tag="bc", name="bcps", bufs=2)
                            nc.tensor.matmul(bc_ps[:], ones64[:], rcp[:],
                                             start=True, stop=True)
                            nc.vector.tensor_copy(bcp[p0:p0 + DH, :], bc_ps[:])
                        # normalize pair -> ctxT (qT tile reuse)
                        nc.vector.tensor_mul(qT[:, pr, :], praw[:].bitcast(F32), bcp[:])

                # ================= Wo + residual + LN1 =================
                if "wo" not in phases:
                    return
                with tc.tile_pool(name="ps_wo", bufs=1, space="PSUM") as ps:
                    wo = [load_w768(WO, l, k, f"wo{k}") for k in range(KT)]
                    for m in range(MT):
                        p_o = ps.tile([128, S], F32, tag="proj", name="po", bufs=3)
                        for k in range(KT):
                            nc.tensor.matmul(p_o[:], wo[k][:, bass.ts(m, 128)],
                                             qT[:, k, :], start=(k == 0),
                                             stop=(k == KT - 1))
                        # y = (psum + bo) + x   (into kTt, reused as y)
                        nc.vector.scalar_tensor_tensor(
                            kTt[:, m, :], p_o[:], bo_t[:, l, m:m + 1],
                            xT[:, m, :].bitcast(F32), op0=ALU.add, op1=ALU.add)
                    layernorm(ps, kTt, g1_t, b1_t, l, attnT)

                # ================= FFN =================
                if "ffn" not in phases:
                    return
                with tc.tile_pool(name="ps_ffn", bufs=1, space="PSUM") as ps:
                    ffo = [ps.tile([128, S], F32, tag="ffo", name=f"ffo{m}", bufs=6)
                           for m in range(MT)]
                    for ko in range(IT):
                        wi_t = wf1pool.tile([128, KT, 128], F32R, tag="wff1",
                                            name=f"wi{ko}")
                        nc.sync.dma_start(out=wi_t[:], in_=WI.ap()[l, ko].bitcast(F32R))
                        wf_t = wpool.tile([128, H], F32R, tag="w768", name=f"wf{ko}")
                        nc.sync.dma_start(out=wf_t[:],
                                          in_=WF.ap()[l, bass.ts(ko, 128), :].bitcast(F32R))
                        p_f = ps.tile([128, S], F32, tag="ff1", name="pf", bufs=2)
                        for k in range(KT):
                            nc.tensor.matmul(p_f[:], wi_t[:, k, :], attnT[:, k, :],
                                             start=(k == 0), stop=(k == KT - 1))
                        ff_t = sb.tile([128, S], F32R, tag="fft", name="fft", bufs=4)
                        nc.scalar.activation(ff_t[:], p_f[:], AF.Gelu,
                                             bias=bi_t[:, l, ko:ko + 1])
                        for m in range(MT):
                            nc.tensor.matmul(ffo[m][:], wf_t[:, bass.ts(m, 128)],
                                             ff_t[:], start=(ko == 0),
                                             stop=(ko == IT - 1))
                    for m in range(MT):
                        # y2 = (ffo + bf) + attnT   (into kTt)
                        nc.vector.scalar_tensor_tensor(
                            kTt[:, m, :], ffo[m][:], bf_t[:, l, m:m + 1],
                            attnT[:, m, :].bitcast(F32), op0=ALU.add, op1=ALU.add)
                with tc.tile_pool(name="ps_ln2", bufs=1, space="PSUM") as ps:
                    layernorm(ps, kTt, g2_t, b2_t, l, xT)

            for _ in range(repeat):
                for l in range(n_layers):
                    layer_body(l)

            nc.sync.dma_start(
                out=OUT.ap().rearrange("(k p) s -> p k s", p=128),
                in_=xT[:].bitcast(F32))

    nc.compile()
    return nc


_CACHE = {}


def get_program(repeat=1, n_layers=L):
    key = (repeat, n_layers)
    if key not in _CACHE:
        _CACHE[key] = build_program(repeat, n_layers)
    return _CACHE[key]


def make_input_maps(inputs):
    """Per-core input maps from the full-batch input dict."""
    hs = np.ascontiguousarray(np.asarray(inputs["hidden_states"], np.float32))
    mask = np.asarray(inputs["attention_mask"], np.float32)
    wi = np.ascontiguousarray(
        np.asarray(inputs["Wi"], np.float32).reshape(L, KT, 128, IT, 128)
        .transpose(0, 3, 2, 1, 4))
    shared = {
        "WQ": np.ascontiguousarray(np.asarray(inputs["Wq"], np.float32)),
        "WK": np.ascontiguousarray(np.asarray(inputs["Wk"], np.float32)),
        "WV": np.ascontiguousarray(np.asarray(inputs["Wv"], np.float32)),
        "WO": np.ascontiguousarray(np.asarray(inputs["Wo"], np.float32)),
        "WI": wi,
        "WF": np.ascontiguousarray(np.asarray(inputs["Wf"], np.float32)),
        "BQ": np.asarray(inputs["bq"], np.float32),
        "BK": np.asarray(inputs["bk"], np.float32),
        "BV": np.asarray(inputs["bv"], np.float32),
        "BO": np.asarray(inputs["bo"], np.float32),
        "BI": np.asarray(inputs["bi"], np.float32),
        "BF": np.asarray(inputs["bf"], np.float32),
        "G1": np.asarray(inputs["ln1_g"], np.float32),
        "B1": np.asarray(inputs["ln1_b"], np.float32),
        "G2": np.asarray(inputs["ln2_g"], np.float32),
        "B2": np.asarray(inputs["ln2_b"], np.float32),
    }
    in_maps = []
    for c in range(B):
        ext = ((1.0 - mask[c]) * -10000.0).astype(np.float32).reshape(ST, 128)
        in_maps.append({
            "XT": np.ascontiguousarray(hs[c].T),
            "EXTM": ext,
            **shared,
        })
    return in_maps


def kernel(**inputs):
    nc = get_program(repeat=1)
    in_maps = make_input_maps(inputs)
    res = run_bass_kernel_spmd(nc, in_maps, list(range(B)))
    out = np.stack([res.results[c]["OUT"].T for c in range(B)], axis=0)
    return out.astype(np.float32)

